# revision 41
# baseline (speedup 1.0000x reference)
"""Trainium2 Bass kernel for DiceLoss (hard-argmax dice, ignore background, mean).

Problem (hardcoded shapes):
  y_true: [16, 512, 512] int32 in [0, 8)
  y_pred: [16, 8, 512, 512] float32
  out   : scalar float32 = mean over classes 1..7 of
          (2*tp + eps) / (2*tp + fp + fn + eps)
  with pred_cls = argmax_c y_pred, one-hot tp/fp/fn sums over all pixels.

Strategy (8 NeuronCores, data-parallel over batch; 2 images per core):
  - Each channel plane is one [128, 2048] tile. y_pred is loaded via SWDGE
    cast-DMA (f32 in HBM -> bf16 in SBUF): HBM read traffic is unchanged but
    every on-chip elementwise op runs in DVE 16-bit perf modes and no
    convert instructions are needed. The per-core stream is a single SWDGE
    FIFO at the HBM bandwidth limit, so everything else is ordered around
    its arrival times: image 0 loads whole planes; image 1 loads two half
    planes so only half a plane of compute trails the final DMA completion.
  - DVE (all bf16, no accum_out so the 2x/4x perf-mode uops stay eligible):
      * 7-op pairwise max tree -> m = max over channels      (2x_1P)
      * pred_c = (ch_c == m) via tensor_tensor is_equal      (2x_1P)
      * gt_c   = (tf == c) via tensor_single_scalar is_equal (4x_2P),
        written strided into a [128, 16, 130] block layout whose col 128
        holds a persistent ones column (memset once). Separate gt tile
        sets per image so image 1's writes never wait on image 0's matmul
        readers (WAR convoy).
  - ScalarE: int32->bf16 label convert; per section a flat copy-with-
    accum_out over the gt block layout that yields the per-partition gt
    counts (host subtracts the constant ones contribution); PSUM evac.
  - TensorE: per class-subtile one matmul with lhsT = pred subtile and
    rhs = [gt subtile | ones] (129 cols) accumulated over subtiles+images:
    diag gives tp, column 128 gives pred counts. Host reads trace + sums.
  - Host: combines the 8 cores' exact-integer f32 partials; dice needs only
    tp and pred_cnt+gt_cnt (denominator = 2tp+fp+fn = pred+gt), formed in
    float32 to match the reference arithmetic.
"""

import numpy as np

EPS = 1e-05

# Problem geometry (hardcoded per the harness contract).
N_CORES = 8
NB = 2            # batch images per core
C = 8             # classes
P = 128           # SBUF partitions
FD = 2048         # free-dim elements per channel plane (512*512 = 128*2048)
NSUB = FD // 128  # 128-wide subtiles per plane for the PE matmuls
BLK = 130         # gt block stride: 128 gt cols + ones col + 1 pad (4B align)

_CACHED_NC = None


def build_bass():
    """Build the Bass kernel (same NEFF for all 8 cores)."""
    from contextlib import ExitStack

    import concourse.bacc as bacc
    import concourse.tile as tile
    from concourse import mybir

    nc = bacc.Bacc(None, target_bir_lowering=False)

    yp = nc.dram_tensor("yp", [NB, C, P, FD], mybir.dt.float32, kind="ExternalInput")
    yt = nc.dram_tensor("yt", [NB, P, FD], mybir.dt.int32, kind="ExternalInput")
    # per class: [128, 129] PSUM accumulator (cross-products + pred colsum).
    mm_out = nc.dram_tensor("mm_out", [7, P, 129], mybir.dt.float32, kind="ExternalOutput")
    # per-partition gt counts: slots 0..6 = (img0, class), 7..13 = (img1,
    # half 0, class), 14..20 = (img1, half 1, class)
    ga_out = nc.dram_tensor("ga_out", [P, 21], mybir.dt.float32, kind="ExternalOutput")

    with tile.TileContext(nc) as tc, ExitStack() as ctx:
        chpool = ctx.enter_context(tc.tile_pool(name="ch", bufs=1))
        tpool = ctx.enter_context(tc.tile_pool(name="tt", bufs=1))
        mpool = ctx.enter_context(tc.tile_pool(name="mx", bufs=2))
        mtmp = ctx.enter_context(tc.tile_pool(name="mtmp", bufs=5))
        predp = ctx.enter_context(tc.tile_pool(name="pred", bufs=5))
        scrp = ctx.enter_context(tc.tile_pool(name="scr", bufs=1))
        accp = ctx.enter_context(tc.tile_pool(name="acc", bufs=1))
        psump = ctx.enter_context(tc.tile_pool(name="psum", bufs=1, space="PSUM"))

        ga_acc = accp.tile([P, 21], mybir.dt.float32, name="ga_acc")
        # fixed per-(image, class) gt tiles in block layout [128, 16, 130]:
        # cols 0:128 = gt mask, col 128 = ones, col 129 = zero pad (so a
        # flat [128, 2080] read sums cleanly). Separate tiles per image so
        # image 1's gt writes never wait on image 0's matmul readers.
        gtsets = [
            [
                accp.tile([P, NSUB, BLK], mybir.dt.bfloat16, name=f"gt{n}_{c}")
                for c in range(1, C)
            ]
            for n in range(NB)
        ]
        psums = [
            psump.tile([P, 129], mybir.dt.float32, name=f"ps{c}", tag=f"ps{c}")
            for c in range(1, C)
        ]

        for gset in gtsets:
            for g in gset:
                nc.vector.memset(g[:, :, 128:129], 1.0)
                nc.vector.memset(g[:, :, 129:130], 0.0)

        HF = FD // 2   # half-plane free dim
        HS = NSUB // 2  # subtiles per half

        # ---- all loads up front: gpsimd queue delivers the casts FIFO in
        # exactly this order; labels ride the concurrent HWDGE queue.
        # Image 0 loads whole planes; image 1 loads half-planes (half-major)
        # so the tail after the last byte is only half a plane's compute. ----
        ch = {}   # (n, c) -> full-plane AP;  (1, c, h) -> half-plane AP
        tf = {}
        for n in range(NB):
            ti = tpool.tile([P, FD], mybir.dt.int32, name="ti", tag=f"ti{n}")
            nc.sync.dma_start(out=ti, in_=yt[n])
            # labels to bf16 (exact for 0..7) on ScalarE
            tfn = tpool.tile([P, FD], mybir.dt.bfloat16, name="tf", tag=f"tf{n}")
            nc.scalar.copy(out=tfn, in_=ti)
            tf[n] = tfn
        for c in range(C):
            tl = chpool.tile([P, FD], mybir.dt.bfloat16, name=f"ch{c}", tag=f"n0ch{c}")
            # SWDGE cast-DMA: f32 HBM -> bf16 SBUF
            nc.gpsimd.dma_start(out=tl, in_=yp[0, c])
            ch[0, c] = tl
        # image 1 sections: two half-planes (quarter-splitting the tail was
        # measured slower — the extra small DMAs lengthen the SWDGE FIFO
        # more than the shorter trailing compute chain saves).
        sections = [(0, HF), (HF, HF)]
        im1 = {}
        for c in range(C):
            im1[c] = chpool.tile([P, FD], mybir.dt.bfloat16, name=f"ch{c}", tag=f"n1ch{c}")
        for si, (off, ln) in enumerate(sections):
            for c in range(C):
                part = im1[c][:, off : off + ln]
                nc.gpsimd.dma_start(out=part, in_=yp[1, c][:, off : off + ln])
                ch[1, c, si] = part

        def emit_gt(gts, slot, tfv, c, s0, ns):
            """gt mask (DVE 4x) + gt count (ScalarE flat copy w/ accum).
            Writes subtile blocks s0..s0+ns of class c's gt tile."""
            g = gts[c - 1]
            gv = g[:, s0 : s0 + ns, 0:128]
            nc.vector.tensor_single_scalar(
                out=gv, in_=tfv, scalar=float(c), op=mybir.AluOpType.is_equal
            )
            scr = scrp.tile([P, NSUB * BLK], mybir.dt.bfloat16, name="scr", tag="scr")
            # flat contiguous read (incl. ones + zero pad; host subtracts
            # the constant 16 per partition per block) keeps ScalarE fast
            nc.scalar.activation(
                out=scr[:, 0 : ns * BLK],
                in_=g[:, s0 : s0 + ns, :].rearrange("p s f -> p (s f)"),
                func=mybir.ActivationFunctionType.Copy,
                accum_out=ga_acc[:, slot : slot + 1],
            )

        def emit_tree(chs, fd, t0, dt):
            """Serial-chain max: the last-arriving channels join closest to
            the root so only 1-2 ops trail the final DMA. The tile_wait_until
            stamps tell the static scheduler the real DMA arrival times, so
            it slots the (early-ready) gt-mask ops into the wait windows
            instead of head-of-line blocking the DVE queue on the tree."""
            t1 = mtmp.tile([P, FD], mybir.dt.bfloat16, name="t1", tag="mt")
            with tc.tile_wait_until(t0 + 2 * dt):
                nc.vector.tensor_max(t1[:, 0:fd], chs[0], chs[1])
            t2 = mtmp.tile([P, FD], mybir.dt.bfloat16, name="t2", tag="mt")
            t12 = mtmp.tile([P, FD], mybir.dt.bfloat16, name="t12", tag="mt")
            with tc.tile_wait_until(t0 + 4 * dt):
                nc.vector.tensor_max(t2[:, 0:fd], chs[2], chs[3])
                nc.vector.tensor_max(t12[:, 0:fd], t1[:, 0:fd], t2[:, 0:fd])
            t3 = mtmp.tile([P, FD], mybir.dt.bfloat16, name="t3", tag="mt")
            t123 = mtmp.tile([P, FD], mybir.dt.bfloat16, name="t123", tag="mt")
            with tc.tile_wait_until(t0 + 6 * dt):
                nc.vector.tensor_max(t3[:, 0:fd], chs[4], chs[5])
                nc.vector.tensor_max(t123[:, 0:fd], t12[:, 0:fd], t3[:, 0:fd])
            t6 = mtmp.tile([P, FD], mybir.dt.bfloat16, name="t6", tag="mt")
            with tc.tile_wait_until(t0 + 7 * dt):
                nc.vector.tensor_max(t6[:, 0:fd], t123[:, 0:fd], chs[6])
            m = mpool.tile([P, FD], mybir.dt.bfloat16, name="m", tag="m")
            with tc.tile_wait_until(t0 + 8 * dt):
                nc.vector.tensor_max(m[:, 0:fd], t6[:, 0:fd], chs[7])
            return m

        def emit_pred_mm(gts, chv, m, c, s0, ns, start, stop):
            pred = predp.tile([P, FD], mybir.dt.bfloat16, name=f"pred{c}", tag="pred")
            predv = pred[:, 0 : ns * 128]
            nc.vector.tensor_tensor(
                out=predv, in0=chv, in1=m, op=mybir.AluOpType.is_equal
            )
            g = gts[c - 1]
            for s in range(ns):
                nc.tensor.matmul(
                    psums[c - 1][:, :],
                    lhsT=predv[:, s * 128 : (s + 1) * 128],
                    rhs=g[:, s0 + s, 0:129],
                    start=(start and s == 0),
                    stop=(stop and s == ns - 1),
                )

        # ---- DVE program, ordered to match arrival times ----
        # All gt masks first: they depend only on the labels (arrive within
        # ~12us on the HWDGE queue) so DVE starts productive work ~6us
        # before the first y_pred channel lands.
        # image 0's gt masks run entirely on ScalarE: gt = relu(1 - |tf - c|)
        # with the gt count fused into the Relu's accum_out (which replaces
        # the separate count-copy, so ScalarE gains only one op per class
        # while DVE sheds all seven tensor_scalar ops).
        bias_neg = accp.tile([P, 7], mybir.dt.float32, name="bias_neg")
        for c in range(1, C):
            nc.gpsimd.memset(bias_neg[:, c - 1 : c], -float(c))
        tf3_0 = tf[0].rearrange("p (s f) -> p s f", s=NSUB)
        for c in range(1, C):
            g = gtsets[0][c - 1]
            gtmp = scrp.tile([P, NSUB, 128], mybir.dt.bfloat16, name="gtmp", tag="gtmp")
            nc.scalar.activation(
                out=gtmp, in_=tf3_0,
                func=mybir.ActivationFunctionType.Abs,
                bias=bias_neg[:, c - 1 : c], scale=1.0,
            )
            nc.scalar.activation(
                out=g[:, :, 0:128], in_=gtmp,
                func=mybir.ActivationFunctionType.Relu,
                bias=1.0, scale=-1.0,
                accum_out=ga_acc[:, c - 1 : c],
            )
        tf3_1 = tf[1].rearrange("p (s f) -> p s f", s=NSUB)
        for si, (off, ln) in enumerate(sections):
            s0, ns = off // 128, ln // 128
            for c in range(1, C):
                emit_gt(gtsets[1], 7 + si * 7 + (c - 1), tf3_1[:, s0 : s0 + ns, :], c, s0, ns)

        # Measured SWDGE FIFO timing (ms): first bytes ~11.5us in, then one
        # 1 MiB full-plane cast every ~2.8us (half planes ~1.4us).
        T0 = 0.0115
        DT_FULL = 0.0028
        DT_HALF = 0.0014

        # image 0: tree, then pred+MM per class.
        m0 = emit_tree([ch[0, c] for c in range(C)], FD, T0 - DT_FULL, DT_FULL)
        for c in range(1, C):
            emit_pred_mm(gtsets[0], ch[0, c], m0, c, 0, NSUB, start=True, stop=False)

        # image 1, by section: tree + pred/MM.
        for si, (off, ln) in enumerate(sections):
            s0, ns = off // 128, ln // 128
            t0 = T0 + 8 * DT_FULL + si * 8 * DT_HALF - DT_HALF
            mh = emit_tree([ch[1, c, si] for c in range(C)], ln, t0, DT_HALF)
            for c in range(1, C):
                emit_pred_mm(
                    gtsets[1], ch[1, c, si], mh[:, 0:ln], c, s0, ns,
                    start=False, stop=(si == len(sections) - 1),
                )

        nc.sync.dma_start(out=ga_out[:], in_=ga_acc)
        for c in range(7):
            pt = accp.tile([P, 129], mybir.dt.float32, name=f"pt{c}", tag=f"pt{c}")
            nc.scalar.copy(out=pt, in_=psums[c])
            nc.sync.dma_start(out=mm_out[c], in_=pt)

    nc.finalize()
    return nc


def _get_bass():
    global _CACHED_NC
    if _CACHED_NC is None:
        _CACHED_NC = build_bass()
    return _CACHED_NC


def make_in_maps(y_true, y_pred):
    yp = np.ascontiguousarray(np.asarray(y_pred, dtype=np.float32))
    yt = np.ascontiguousarray(np.asarray(y_true, dtype=np.int32))
    in_maps = []
    for i in range(N_CORES):
        yps = np.ascontiguousarray(yp[NB * i : NB * (i + 1)]).reshape(NB, C, P, FD)
        yts = np.ascontiguousarray(yt[NB * i : NB * (i + 1)]).reshape(NB, P, FD)
        in_maps.append({"yp": yps, "yt": yts})
    return in_maps


def epilogue(results):
    """Combine the 8 cores' partial sums into the final dice mean (float32,
    mirroring the reference arithmetic)."""
    tp = np.zeros(7, dtype=np.float64)
    pred_cnt = np.zeros(7, dtype=np.float64)
    gt_cnt = np.zeros(7, dtype=np.float64)
    for r in results:
        mm = np.asarray(r["mm_out"], dtype=np.float64)  # [7, P, 129]
        tp += np.trace(mm[:, :, :128], axis1=1, axis2=2)
        pred_cnt += mm[:, :, 128].sum(axis=1)
        ga = np.asarray(r["ga_out"], dtype=np.float64).sum(axis=0)  # [21]
        # img0 slots (0..6) are exact (ScalarE relu accum sums only the
        # mask); img1 half slots are flat copies that include one
        # ones-column entry per block per partition: (8 + 8) * P per class
        gt_cnt += ga[0:7] + ga[7:14] + ga[14:21] - 16 * P

    tp32 = tp.astype(np.float32)
    fp32_ = (pred_cnt - tp).astype(np.float32)
    fn32 = (gt_cnt - tp).astype(np.float32)
    eps = np.float32(EPS)
    two = np.float32(2.0)
    dice = (two * tp32 + eps) / (two * tp32 + fp32_ + fn32 + eps)
    return np.asarray(np.mean(dice, dtype=np.float32), dtype=np.float32)


def kernel(**inputs):
    from concourse.bass_utils import run_bass_kernel_spmd

    nc = _get_bass()
    in_maps = make_in_maps(inputs["y_true"], inputs["y_pred"])
    res = run_bass_kernel_spmd(nc, in_maps, core_ids=list(range(N_CORES)))
    return epilogue(res.results)


if __name__ == "__main__":
    # smoke test with random data
    rng = np.random.default_rng(0)
    y_true = rng.integers(0, C, size=(16, 512, 512)).astype(np.int32)
    y_pred = rng.standard_normal((16, C, 512, 512)).astype(np.float32)
    out = kernel(y_true=y_true, y_pred=y_pred)
    print("kernel output:", out)


# revision 44
# speedup vs baseline: 1.3864x; 1.3864x over previous
"""Trainium2 Bass kernel for DiceLoss (hard-argmax dice, ignore background, mean).

Problem (hardcoded shapes):
  y_true: [16, 512, 512] int32 in [0, 8)
  y_pred: [16, 8, 512, 512] float32
  out   : scalar float32 = mean over classes 1..7 of
          (2*tp + eps) / (2*tp + fp + fn + eps)
  with pred_cls = argmax_c y_pred, one-hot tp/fp/fn sums over all pixels.

Strategy (8 NeuronCores, data-parallel over batch; 2 images per core):
  - Each channel plane is one [128, 2048] tile. y_pred is loaded via SWDGE
    cast-DMA (f32 in HBM -> bf16 in SBUF): HBM read traffic is unchanged but
    every on-chip elementwise op runs in DVE 16-bit perf modes and no
    convert instructions are needed. The per-core stream is a single SWDGE
    FIFO at the HBM bandwidth limit, so everything else is ordered around
    its arrival times: image 0 loads whole planes; image 1 loads two half
    planes so only half a plane of compute trails the final DMA completion.
  - DVE (all bf16, no accum_out so the 2x/4x perf-mode uops stay eligible):
      * 7-op pairwise max tree -> m = max over channels      (2x_1P)
      * pred_c = (ch_c == m) via tensor_tensor is_equal      (2x_1P)
      * gt_c   = (tf == c) via tensor_single_scalar is_equal (4x_2P),
        written strided into a [128, 16, 130] block layout whose col 128
        holds a persistent ones column (memset once). Separate gt tile
        sets per image so image 1's writes never wait on image 0's matmul
        readers (WAR convoy).
  - ScalarE: int32->bf16 label convert; per section a flat copy-with-
    accum_out over the gt block layout that yields the per-partition gt
    counts (host subtracts the constant ones contribution); PSUM evac.
  - TensorE: per class-subtile one matmul with lhsT = pred subtile and
    rhs = [gt subtile | ones] (129 cols) accumulated over subtiles+images:
    diag gives tp, column 128 gives pred counts. Host reads trace + sums.
  - Host: combines the 8 cores' exact-integer f32 partials; dice needs only
    tp and pred_cnt+gt_cnt (denominator = 2tp+fp+fn = pred+gt), formed in
    float32 to match the reference arithmetic.
"""

import numpy as np

EPS = 1e-05

# Problem geometry (hardcoded per the harness contract).
N_CORES = 8
NB = 2            # batch images per core
C = 8             # classes
P = 128           # SBUF partitions
FD = 2048         # free-dim elements per channel plane (512*512 = 128*2048)
NSUB = FD // 128  # 128-wide subtiles per plane for the PE matmuls
BLK = 130         # gt block stride: 128 gt cols + ones col + 1 pad (4B align)

_CACHED_NC = None


def build_bass():
    """Build the Bass kernel (same NEFF for all 8 cores)."""
    from contextlib import ExitStack

    import concourse.bacc as bacc
    import concourse.tile as tile
    from concourse import mybir

    nc = bacc.Bacc(None, target_bir_lowering=False)

    yp = nc.dram_tensor("yp", [NB, C, P, FD], mybir.dt.float32, kind="ExternalInput")
    yt = nc.dram_tensor("yt", [NB, P, FD], mybir.dt.int32, kind="ExternalInput")
    # per class: [128, 129] PSUM accumulator (cross-products + pred colsum).
    mm_out = nc.dram_tensor("mm_out", [7, P, 129], mybir.dt.float32, kind="ExternalOutput")
    # per-partition gt counts: slots 0..6 = (img0, class), 7..13 = (img1,
    # half 0, class), 14..20 = (img1, half 1, class)
    ga_out = nc.dram_tensor("ga_out", [P, 21], mybir.dt.float32, kind="ExternalOutput")

    with tile.TileContext(nc) as tc, ExitStack() as ctx:
        chpool = ctx.enter_context(tc.tile_pool(name="ch", bufs=1))
        tpool = ctx.enter_context(tc.tile_pool(name="tt", bufs=1))
        mpool = ctx.enter_context(tc.tile_pool(name="mx", bufs=2))
        mtmp = ctx.enter_context(tc.tile_pool(name="mtmp", bufs=6))
        predp = ctx.enter_context(tc.tile_pool(name="pred", bufs=5))
        scrp = ctx.enter_context(tc.tile_pool(name="scr", bufs=1))
        accp = ctx.enter_context(tc.tile_pool(name="acc", bufs=1))
        psump = ctx.enter_context(tc.tile_pool(name="psum", bufs=1, space="PSUM"))

        ga_acc = accp.tile([P, 21], mybir.dt.float32, name="ga_acc")
        # fixed per-(image, class) gt tiles in block layout [128, 16, 130]:
        # cols 0:128 = gt mask, col 128 = ones, col 129 = zero pad (so a
        # flat [128, 2080] read sums cleanly). Separate tiles per image so
        # image 1's gt writes never wait on image 0's matmul readers.
        gtsets = [
            [
                accp.tile([P, NSUB, BLK], mybir.dt.bfloat16, name=f"gt{n}_{c}")
                for c in range(1, C)
            ]
            for n in range(NB)
        ]
        psums = [
            psump.tile([P, 129], mybir.dt.float32, name=f"ps{c}", tag=f"ps{c}")
            for c in range(1, C)
        ]

        for gset in gtsets:
            for g in gset:
                nc.vector.memset(g[:, :, 128:129], 1.0)
                nc.vector.memset(g[:, :, 129:130], 0.0)

        HF = FD // 2   # half-plane free dim
        HS = NSUB // 2  # subtiles per half

        # ---- all loads up front: gpsimd queue delivers the casts FIFO in
        # exactly this order; labels ride the concurrent HWDGE queue.
        # Image 0 loads whole planes; image 1 loads half-planes (half-major)
        # so the tail after the last byte is only half a plane's compute. ----
        ch = {}   # (n, c) -> full-plane AP;  (1, c, h) -> half-plane AP
        tf = {}
        for n in range(NB):
            ti = tpool.tile([P, FD], mybir.dt.int32, name="ti", tag=f"ti{n}")
            nc.sync.dma_start(out=ti, in_=yt[n])
            # labels to bf16 (exact for 0..7) on ScalarE
            tfn = tpool.tile([P, FD], mybir.dt.bfloat16, name="tf", tag=f"tf{n}")
            nc.scalar.copy(out=tfn, in_=ti)
            tf[n] = tfn
        for c in range(C):
            tl = chpool.tile([P, FD], mybir.dt.bfloat16, name=f"ch{c}", tag=f"n0ch{c}")
            # SWDGE cast-DMA: f32 HBM -> bf16 SBUF
            nc.gpsimd.dma_start(out=tl, in_=yp[0, c])
            ch[0, c] = tl
        # image 1 sections: two half-planes (quarter-splitting the tail was
        # measured slower — the extra small DMAs lengthen the SWDGE FIFO
        # more than the shorter trailing compute chain saves).
        sections = [(0, HF), (HF, HF)]
        im1 = {}
        for c in range(C):
            im1[c] = chpool.tile([P, FD], mybir.dt.bfloat16, name=f"ch{c}", tag=f"n1ch{c}")
        for si, (off, ln) in enumerate(sections):
            for c in range(C):
                part = im1[c][:, off : off + ln]
                nc.gpsimd.dma_start(out=part, in_=yp[1, c][:, off : off + ln])
                ch[1, c, si] = part

        def emit_gt(gts, slot, tfv, c, s0, ns):
            """gt mask (DVE 4x) + gt count (ScalarE flat copy w/ accum).
            Writes subtile blocks s0..s0+ns of class c's gt tile."""
            g = gts[c - 1]
            gv = g[:, s0 : s0 + ns, 0:128]
            nc.vector.tensor_single_scalar(
                out=gv, in_=tfv, scalar=float(c), op=mybir.AluOpType.is_equal
            )
            scr = scrp.tile([P, NSUB * BLK], mybir.dt.bfloat16, name="scr", tag="scr")
            # flat contiguous read (incl. ones + zero pad; host subtracts
            # the constant 16 per partition per block) keeps ScalarE fast
            nc.scalar.activation(
                out=scr[:, 0 : ns * BLK],
                in_=g[:, s0 : s0 + ns, :].rearrange("p s f -> p (s f)"),
                func=mybir.ActivationFunctionType.Copy,
                accum_out=ga_acc[:, slot : slot + 1],
            )

        def emit_tree(chs, fd, t0, dt):
            """Serial-chain max: the last-arriving channels join closest to
            the root so only 1-2 ops trail the final DMA. The tile_wait_until
            stamps tell the static scheduler the real DMA arrival times, so
            it slots the (early-ready) gt-mask ops into the wait windows
            instead of head-of-line blocking the DVE queue on the tree."""
            t1 = mtmp.tile([P, FD], mybir.dt.bfloat16, name="t1", tag="mt")
            with tc.tile_wait_until(t0 + 2 * dt):
                nc.vector.tensor_max(t1[:, 0:fd], chs[0], chs[1])
            t2 = mtmp.tile([P, FD], mybir.dt.bfloat16, name="t2", tag="mt")
            t12 = mtmp.tile([P, FD], mybir.dt.bfloat16, name="t12", tag="mt")
            with tc.tile_wait_until(t0 + 4 * dt):
                nc.vector.tensor_max(t2[:, 0:fd], chs[2], chs[3])
                nc.vector.tensor_max(t12[:, 0:fd], t1[:, 0:fd], t2[:, 0:fd])
            t3 = mtmp.tile([P, FD], mybir.dt.bfloat16, name="t3", tag="mt")
            t123 = mtmp.tile([P, FD], mybir.dt.bfloat16, name="t123", tag="mt")
            with tc.tile_wait_until(t0 + 6 * dt):
                nc.vector.tensor_max(t3[:, 0:fd], chs[4], chs[5])
                nc.vector.tensor_max(t123[:, 0:fd], t12[:, 0:fd], t3[:, 0:fd])
            t6 = mtmp.tile([P, FD], mybir.dt.bfloat16, name="t6", tag="mt")
            with tc.tile_wait_until(t0 + 7 * dt):
                nc.vector.tensor_max(t6[:, 0:fd], t123[:, 0:fd], chs[6])
            m = mpool.tile([P, FD], mybir.dt.bfloat16, name="m", tag="m")
            with tc.tile_wait_until(t0 + 8 * dt):
                nc.vector.tensor_max(m[:, 0:fd], t6[:, 0:fd], chs[7])
            return m

        def emit_pred_mm(gts, chv, m, c, s0, ns, start, stop):
            pred = predp.tile([P, FD], mybir.dt.bfloat16, name=f"pred{c}", tag="pred")
            predv = pred[:, 0 : ns * 128]
            nc.vector.tensor_tensor(
                out=predv, in0=chv, in1=m, op=mybir.AluOpType.is_equal
            )
            g = gts[c - 1]
            for s in range(ns):
                nc.tensor.matmul(
                    psums[c - 1][:, :],
                    lhsT=predv[:, s * 128 : (s + 1) * 128],
                    rhs=g[:, s0 + s, 0:129],
                    start=(start and s == 0),
                    stop=(stop and s == ns - 1),
                )

        # ---- DVE program, ordered to match arrival times ----
        # All gt masks first: they depend only on the labels (arrive within
        # ~12us on the HWDGE queue) so DVE starts productive work ~6us
        # before the first y_pred channel lands.
        # image 0's gt masks run entirely on ScalarE: gt = relu(1 - |tf - c|)
        # with the gt count fused into the Relu's accum_out (which replaces
        # the separate count-copy, so ScalarE gains only one op per class
        # while DVE sheds all seven tensor_scalar ops).
        tf3_0 = tf[0].rearrange("p (s f) -> p s f", s=NSUB)
        for c in range(1, C):
            emit_gt(gtsets[0], c - 1, tf3_0, c, 0, NSUB)
        tf3_1 = tf[1].rearrange("p (s f) -> p s f", s=NSUB)
        for si, (off, ln) in enumerate(sections):
            s0, ns = off // 128, ln // 128
            for c in range(1, C):
                emit_gt(gtsets[1], 7 + si * 7 + (c - 1), tf3_1[:, s0 : s0 + ns, :], c, s0, ns)

        # Measured SWDGE FIFO timing (ms): first bytes ~11.5us in, then one
        # 1 MiB full-plane cast every ~2.8us (half planes ~1.4us).
        T0 = 0.0115
        DT_FULL = 0.0028
        DT_HALF = 0.0014

        # image 0: tree, then pred+MM per class.
        m0 = emit_tree([ch[0, c] for c in range(C)], FD, T0 - DT_FULL, DT_FULL)
        for c in range(1, C):
            emit_pred_mm(gtsets[0], ch[0, c], m0, c, 0, NSUB, start=True, stop=False)

        # image 1, by section: tree + pred/MM.
        for si, (off, ln) in enumerate(sections):
            s0, ns = off // 128, ln // 128
            t0 = T0 + 8 * DT_FULL + si * 8 * DT_HALF - DT_HALF
            mh = emit_tree([ch[1, c, si] for c in range(C)], ln, t0, DT_HALF)
            for c in range(1, C):
                emit_pred_mm(
                    gtsets[1], ch[1, c, si], mh[:, 0:ln], c, s0, ns,
                    start=False, stop=(si == len(sections) - 1),
                )

        nc.sync.dma_start(out=ga_out[:], in_=ga_acc)
        for c in range(7):
            pt = accp.tile([P, 129], mybir.dt.float32, name=f"pt{c}", tag=f"pt{c}")
            nc.scalar.copy(out=pt, in_=psums[c])
            nc.sync.dma_start(out=mm_out[c], in_=pt)

    nc.finalize()
    return nc


def _get_bass():
    global _CACHED_NC
    if _CACHED_NC is None:
        _CACHED_NC = build_bass()
    return _CACHED_NC


def make_in_maps(y_true, y_pred):
    yp = np.ascontiguousarray(np.asarray(y_pred, dtype=np.float32))
    yt = np.ascontiguousarray(np.asarray(y_true, dtype=np.int32))
    in_maps = []
    for i in range(N_CORES):
        yps = np.ascontiguousarray(yp[NB * i : NB * (i + 1)]).reshape(NB, C, P, FD)
        yts = np.ascontiguousarray(yt[NB * i : NB * (i + 1)]).reshape(NB, P, FD)
        in_maps.append({"yp": yps, "yt": yts})
    return in_maps


def epilogue(results):
    """Combine the 8 cores' partial sums into the final dice mean (float32,
    mirroring the reference arithmetic)."""
    tp = np.zeros(7, dtype=np.float64)
    pred_cnt = np.zeros(7, dtype=np.float64)
    gt_cnt = np.zeros(7, dtype=np.float64)
    for r in results:
        mm = np.asarray(r["mm_out"], dtype=np.float64)  # [7, P, 129]
        tp += np.trace(mm[:, :, :128], axis1=1, axis2=2)
        pred_cnt += mm[:, :, 128].sum(axis=1)
        ga = np.asarray(r["ga_out"], dtype=np.float64).sum(axis=0)  # [21]
        # each slot's flat accum includes one ones-column entry per block
        # per partition: 16 (img0) + 8 + 8 (img1 halves) = 32*P total
        gt_cnt += ga[0:7] + ga[7:14] + ga[14:21] - 2 * 16 * P

    tp32 = tp.astype(np.float32)
    fp32_ = (pred_cnt - tp).astype(np.float32)
    fn32 = (gt_cnt - tp).astype(np.float32)
    eps = np.float32(EPS)
    two = np.float32(2.0)
    dice = (two * tp32 + eps) / (two * tp32 + fp32_ + fn32 + eps)
    return np.asarray(np.mean(dice, dtype=np.float32), dtype=np.float32)


def kernel(**inputs):
    from concourse.bass_utils import run_bass_kernel_spmd

    nc = _get_bass()
    in_maps = make_in_maps(inputs["y_true"], inputs["y_pred"])
    res = run_bass_kernel_spmd(nc, in_maps, core_ids=list(range(N_CORES)))
    return epilogue(res.results)


if __name__ == "__main__":
    # smoke test with random data
    rng = np.random.default_rng(0)
    y_true = rng.integers(0, C, size=(16, 512, 512)).astype(np.int32)
    y_pred = rng.standard_normal((16, C, 512, 512)).astype(np.float32)
    out = kernel(y_true=y_true, y_pred=y_pred)
    print("kernel output:", out)
